# revision 65
# baseline (speedup 1.0000x reference)
"""Adaptive block-sparse attention (train fwd) on 8 Trainium2 NeuronCores.

Reference semantics (B=1, H=12, S=4096, D=128, BLOCK=128, NUM_KEEP=32):
  1. per (b,h): sample 32 tokens/block from q and k (shared intra-block offsets)
  2. pooled attention estimate -> block scores pool[qb, kb]  (32x32 per head)
  3. energy threshold (0.95) on descending-sorted block scores -> keep top-n
     blocks per q-block row, n clipped to [1, 5]
  4. block-sparse attention with that mask.

Sharding: 24 units = (head, half of 2048 q rows); core c owns units 3c..3c+2,
which span exactly heads {floor(3c/2), floor(3c/2)+1}. Each core receives the
2 heads' combined K/V block tensor, per-unit Q^T, and f32 pre-sampled
sq^T/sk^T for the pooling estimate (pooling must stay f32: the rank-5/6 pool
gaps go down to ~1e-5 relative). All pooling, mask generation, top-k
selection and the block-sparse attention run on-device; the host lays out
shards and divides by the returned softmax-denominator column on unshard.

Combined K/V layout per (head-slot j, block b), stride BSC=264 bf16 cols:
  cols 0..127   K^T block   kv[p=d, c]     = k[h, b*128+c, p]
  cols 128..255 V block     kv[p=k, 128+c] = v[h, b*128+p, c]
  col  256      ones        (denominator accumulated by the PV matmul)
  block 32 is all zero: masked slots aim their V offset at it, so exp of the
  (real) scores times zero V and zero ones-col adds nothing to numerator or
  denominator.

Engine plan (per q-block: 5 combined-block gathers + 5 S^T matmuls + one
[128, 640] exp + 5 accumulating PV matmuls + PSUM evacuation):
  SP     gather slot 0 (HWDGE DMA), batched output DMAs, input DMAs
  GpSimd gather slot 1 (SWDGE DMA - data moves on the SDMA engines, so no
         SBUF-port contention with DVE)
  DVE    gather slots 2-4 (dynamic-offset tensor_copy), half the PSUM
         evacuations, pooling reductions + mask ops, the offset transpose
  ACT    exp, plus half the PSUM evacuations
  PE     all matmuls (both attention operands are static slices of the
         gathered combined block, so PE needs no registers at all);
         per-unit offsets reach SP/GpSimd/DVE via one bulk 16-value
         register load per gather engine row

The pooling/mask chain for unit u+1 is emitted interleaved between unit u's
attention q-blocks so its long serial latency hides under attention work.
"""
import os
import sys

sys.path.insert(0, "/opt/trn_rl_repo")

import numpy as np
import ml_dtypes

import concourse.bass as bass
import concourse.bacc as bacc
import concourse.mybir as mybir
from concourse import tile
from concourse.bass_utils import run_bass_kernel_spmd

B, H, S, D = 1, 12, 4096, 128
BLOCK = 128
NUM_KEEP = 32
NB = S // BLOCK            # 32 k-blocks per head
E_THR = 0.95
NCORES = 8
UPC = 3                    # units per core
QH = 2048                  # q rows per unit (half head)
NQB = QH // BLOCK          # 16 q-blocks per unit
NS = 5                     # retained-slot count (MAX_RETAIN)
BSC = 264                  # combined block stride: 128 K + 128 V + 1 ones + 7 pad
HW_ = (NB + 1) * BSC       # combined width per head (incl. zero block)
KVW = 2 * HW_              # two heads
SCALE = float(1.0 / np.sqrt(D))

_CACHE = {}


def _build_nc():
    f32 = mybir.dt.float32
    bf16 = mybir.dt.bfloat16
    i32 = mybir.dt.int32
    u32 = mybir.dt.uint32
    EXP = mybir.ActivationFunctionType.Exp
    OP = mybir.AluOpType
    SP = mybir.EngineType.SP
    ACT = mybir.EngineType.Activation
    DVE = mybir.EngineType.DVE
    POOL = mybir.EngineType.Pool
    PE = mybir.EngineType.PE

    nc = bacc.Bacc("TRN2", target_bir_lowering=False, debug=False,
                   num_devices=NCORES)

    qT_d = nc.dram_tensor("qT", [UPC, D, QH], bf16, kind="ExternalInput")
    kv_d = nc.dram_tensor("kv", [D, KVW], bf16, kind="ExternalInput")
    smp_d = nc.dram_tensor("smp", [UPC, D, (NQB + NB) * NUM_KEEP], f32, kind="ExternalInput")
    qbsel_d = nc.dram_tensor("qbsel", [D, 4 * NQB], f32, kind="ExternalInput")
    misc_d = nc.dram_tensor("misc", [NQB, NS + UPC], f32, kind="ExternalInput")
    out_d = nc.dram_tensor("out", [UPC, QH, BLOCK + 1], f32, kind="ExternalOutput")

    with tile.TileContext(nc) as tc:
        with (
            tc.tile_pool(name="const", bufs=1) as cpool,
            tc.tile_pool(name="unit", bufs=3) as upool,
            tc.tile_pool(name="pwork", bufs=3) as pwork,
            tc.tile_pool(name="mask", bufs=2) as mpool,
            tc.tile_pool(name="kg", bufs=10) as kgpool,
            tc.tile_pool(name="pt", bufs=4) as ptpool,
            tc.tile_pool(name="big", bufs=2, space="PSUM") as bigp,
            tc.tile_pool(name="po", bufs=2, space="PSUM") as pop,
            tc.tile_pool(name="pp", bufs=1, space="PSUM") as ppp,
        ):
            qbsel = cpool.tile([D, 4 * NQB], f32)
            nc.sync.dma_start(qbsel, qbsel_d[:, :])
            misc = cpool.tile([NQB, NS + UPC], f32)
            nc.sync.dma_start(misc, misc_d[:, :])
            iota5 = misc[:, 0:NS]
            cbase = misc[:, NS:NS + UPC]
            zero8 = cpool.tile([NQB, 8], f32)
            nc.vector.memset(zero8, 0.0)
            warm = cpool.tile([NQB, 8], f32)
            nc.scalar.activation(warm, zero8, EXP, scale=1.0)
            kv = cpool.tile([D, KVW], bf16)

            unit_state = {}

            def emit_pool_steps(u):
                """Generator of pooling/mask emission steps for unit u."""
                st = {}
                unit_state[u] = st

                def load():
                    st["qT"] = upool.tile([D, QH], bf16, tag="qT", name="qTu")
                    nc.sync.dma_start(st["qT"], qT_d[u, :, :])
                    smp = upool.tile([D, (NQB + NB) * NUM_KEEP], f32, tag="smp", name="smpu")
                    nc.sync.dma_start(smp, smp_d[u, :, :])
                    st["smp"] = smp
                    st["poolps"] = ppp.tile([NQB, NB], f32, tag="pp", name="poolps")
                yield load

                for t in range(4):
                    for half in range(2):
                        def chunk(t=t, half=half):
                            smp = st["smp"]
                            sqT = smp[:, 0:NQB * NUM_KEEP]
                            skT = smp[:, NQB * NUM_KEEP:(NQB + NB) * NUM_KEEP]
                            ps = bigp.tile([D, 512], f32, tag="big", name="psc")
                            nc.tensor.matmul(
                                ps, sqT[:, t * 128:(t + 1) * 128],
                                skT[:, half * 512:(half + 1) * 512],
                                start=True, stop=True)
                            E = pwork.tile([D, 512], f32, tag="E")
                            nc.scalar.activation(E, ps, EXP, scale=SCALE)
                            if half == 0:
                                st["W"] = pwork.tile([D, NB], f32, tag="W", name="Wt")
                            nc.vector.reduce_sum(
                                st["W"][:, half * 16:(half + 1) * 16],
                                E.rearrange("p (a b) -> p a b", a=16),
                                axis=mybir.AxisListType.X)
                        yield chunk

                    def tfin(t=t):
                        W = st["W"]
                        T = pwork.tile([D, 1], f32, tag="T")
                        nc.vector.reduce_sum(T, W, axis=mybir.AxisListType.X)
                        R = pwork.tile([D, 1], f32, tag="R")
                        nc.vector.reciprocal(R, T)
                        Wn = pwork.tile([D, NB], f32, tag="Wn")
                        nc.vector.tensor_scalar_mul(Wn, W, R[:, 0:1])
                        nc.tensor.matmul(
                            st["poolps"], qbsel[:, t * NQB:(t + 1) * NQB], Wn,
                            start=(t == 0), stop=(t == 3))
                    yield tfin

                def mask_fin():
                    poolps = st["poolps"]
                    pool_sb = mpool.tile([NQB, NB], f32, tag="pool_sb")
                    nc.vector.tensor_copy(pool_sb, poolps)
                    T16 = mpool.tile([NQB, 1], f32, tag="T16")
                    nc.vector.reduce_sum(T16, pool_sb, axis=mybir.AxisListType.X)
                    thr = mpool.tile([NQB, 1], f32, tag="thr")
                    nc.vector.tensor_scalar_mul(thr, T16, E_THR)
                    m8 = mpool.tile([NQB, 8], f32, tag="m8")
                    i8 = mpool.tile([NQB, 8], u32, tag="i8")
                    nc.vector.max_with_indices(m8, i8, pool_sb)
                    cum = mpool.tile([NQB, 8], f32, tag="cum")
                    nc.vector.tensor_tensor_scan(cum, m8, zero8, 0.0,
                                                 OP.add, OP.add)
                    lt = mpool.tile([NQB, 8], f32, tag="lt")
                    nc.vector.tensor_scalar(lt, cum, thr[:, 0:1], None,
                                            op0=OP.is_lt)
                    cnt = mpool.tile([NQB, 1], f32, tag="cnt")
                    nc.vector.reduce_sum(cnt, lt, axis=mybir.AxisListType.X)
                    nkeep = mpool.tile([NQB, 1], f32, tag="nkeep")
                    nc.vector.tensor_scalar(nkeep, cnt, 1.0, float(NS),
                                            op0=OP.max, op1=OP.min)
                    slt = mpool.tile([NQB, NS], f32, tag="slt")
                    nc.vector.tensor_scalar(slt, iota5, nkeep[:, 0:1], None,
                                            op0=OP.is_lt)
                    idx5f = mpool.tile([NQB, NS], f32, tag="idx5f")
                    nc.vector.tensor_copy(idx5f, i8[:, 0:NS])
                    # cofs = 264*slt*(idx-32) + (kbase[u] + 32*264): kept slots
                    # hit the real block, masked ones the all-zero block 32
                    a5 = mpool.tile([NQB, NS], f32, tag="a5")
                    nc.vector.tensor_scalar(a5, idx5f, float(NB), None,
                                            op0=OP.subtract)
                    nc.vector.tensor_mul(a5, a5, slt)
                    cofs = mpool.tile([NQB, NS], f32, tag="cofs")
                    nc.vector.tensor_scalar_mul(cofs, a5, float(BSC))
                    nc.vector.tensor_add(
                        cofs, cofs, cbase[:, u:u + 1].to_broadcast([NQB, NS]))
                    ofs = mpool.tile([32, 32], i32, tag="ofs")
                    nc.vector.memset(ofs, 0)
                    nc.vector.tensor_copy(ofs[0:NQB, 0:NS], cofs)
                    ofsT = mpool.tile([32, 32], i32, tag="ofsT")
                    nc.vector.transpose(ofsT, ofs)
                    kvals = []
                    for s, engs in ((0, [SP]), (1, [POOL]), (2, [DVE]),
                                    (3, [DVE]), (4, [DVE])):
                        _, vv = nc.values_load_multi_w_load_instructions(
                            ofsT[s:s + 1, 0:NQB], engines=engs,
                            min_val=0, max_val=KVW - BSC,
                            skip_runtime_bounds_check=True)
                        kvals.append(vv)
                    st["kvals"] = kvals
                yield mask_fin

            def attn_a(u, qb):
                # gathers + S^T matmuls + exp; PV is deferred one q-block so
                # PE's in-order stream never waits on exp
                st = unit_state[u]
                kvals = st["kvals"]
                kgt = []
                for s in range(NS):
                    kg = kgpool.tile([D, BSC], bf16, tag=f"kg{s}")
                    kgt.append(kg)
                    src = kv[:, bass.ds(kvals[s][qb], BSC)]
                    if s == 0:
                        nc.sync.dma_start(kg, src)
                    elif s == 1:
                        nc.gpsimd.dma_start(kg, src)
                    else:
                        nc.vector.tensor_copy(kg, src)
                ps2 = bigp.tile([D, NS * BLOCK], f32, tag="big")
                for s in range(NS):
                    nc.tensor.matmul(
                        ps2[:, s * BLOCK:(s + 1) * BLOCK], kgt[s][:, 0:BLOCK],
                        st["qT"][:, qb * BLOCK:(qb + 1) * BLOCK],
                        start=True, stop=True)
                pT = ptpool.tile([D, NS * BLOCK], bf16, tag="pT")
                nc.scalar.activation(pT, ps2, EXP, scale=SCALE)
                return pT, kgt

            def attn_b(u, qb, pT, kgt):
                st = unit_state[u]
                po = pop.tile([D, BLOCK + 1], f32, tag="po")
                for s in range(NS):
                    nc.tensor.matmul(
                        po, pT[:, s * BLOCK:(s + 1) * BLOCK],
                        kgt[s][:, BLOCK:2 * BLOCK + 1],
                        start=(s == 0), stop=(s == NS - 1))
                # stage 4 q-blocks per output DMA
                j = qb % 4
                if j == 0:
                    st["outsb"] = ptpool.tile([D, 4, BLOCK + 1], f32, tag="outsb", name="outsb")
                nc.vector.tensor_copy(st["outsb"][:, j, :], po)
                if j == 3:
                    nc.sync.dma_start(
                        out_d[u, (qb - 3) * BLOCK:(qb + 1) * BLOCK, :]
                        .rearrange("(a p) c -> p a c", p=BLOCK),
                        st["outsb"])

            # unit 0's pooling/mask runs upfront (its small input DMAs are
            # issued before the big kv load so the pooling chain starts
            # immediately); unit u+1's pooling is interleaved into unit u's
            # attention q-blocks.
            for i, step in enumerate(emit_pool_steps(0)):
                step()
                if i == 0:
                    nc.sync.dma_start(kv, kv_d[:, :])
            for u in range(UPC):
                nxt = iter(emit_pool_steps(u + 1)) if u + 1 < UPC else None
                pend = None
                for qb in range(NQB):
                    a = attn_a(u, qb)
                    if pend is not None:
                        attn_b(u, qb - 1, *pend)
                    pend = a
                    if nxt is not None:
                        for _ in range(2):
                            step = next(nxt, None)
                            if step is not None:
                                step()
                attn_b(u, NQB - 1, *pend)

    nc.compile()
    return nc


def _shard_inputs(q, k, v, idx_q, idx_k):
    """Build the 8 per-core input maps."""
    bf16 = ml_dtypes.bfloat16
    q = np.asarray(q, np.float32)[0]          # [H, S, D]
    k = np.asarray(k, np.float32)[0]
    v = np.asarray(v, np.float32)[0]
    idx_q = np.asarray(idx_q)[0]              # [H, NUM_KEEP]
    idx_k = np.asarray(idx_k)[0]

    qbsel = np.zeros((D, 4 * NQB), np.float32)
    for t in range(4):
        for s in range(D):
            qbsel[s, t * NQB + t * 4 + s // NUM_KEEP] = 1.0
    iota5 = np.tile(np.arange(NS, dtype=np.float32), (NQB, 1))

    in_maps = []
    for c in range(NCORES):
        h_lo = (3 * c) // 2
        units = [(uu // 2, uu % 2) for uu in range(3 * c, 3 * c + 3)]

        # combined K/V blocks, stride BSC; block NB left all-zero
        kvc = np.zeros((D, 2, NB + 1, BSC), np.float32)
        for j, h in enumerate((h_lo, h_lo + 1)):
            kvc[:, j, :NB, :BLOCK] = k[h].reshape(NB, BLOCK, D).transpose(2, 0, 1)
            kvc[:, j, :NB, BLOCK:2 * BLOCK] = (
                v[h].reshape(NB, BLOCK, D).transpose(1, 0, 2))
            kvc[:, j, :NB, 2 * BLOCK] = 1.0
        kvc = kvc.reshape(D, KVW).astype(bf16)

        qT = np.stack([q[h, hf * QH:(hf + 1) * QH].T for h, hf in units]
                      ).astype(bf16)                               # [3, D, QH]

        sqT, skT = [], []
        for h, hf in units:
            sq = q[h, hf * QH:(hf + 1) * QH].reshape(NQB, BLOCK, D)[
                :, idx_q[h], :]                                    # [16, 32, D]
            sqT.append(sq.transpose(2, 0, 1).reshape(D, NQB * NUM_KEEP))
            sk = k[h].reshape(NB, BLOCK, D)[:, idx_k[h], :]        # [32, 32, D]
            skT.append(sk.transpose(2, 0, 1).reshape(D, NB * NUM_KEEP))
        smp = np.concatenate(
            [np.stack(sqT), np.stack(skT)], axis=2).astype(np.float32)

        uslot = np.array([h - h_lo for h, _ in units], np.float32)
        cbase = np.tile(uslot * HW_ + NB * BSC, (NQB, 1)).astype(np.float32)
        misc = np.concatenate([iota5, cbase], axis=1).astype(np.float32)

        in_maps.append({
            "qT": qT, "kv": kvc, "smp": smp, "qbsel": qbsel, "misc": misc,
        })
    return in_maps


def kernel(q, k, v, idx_q, idx_k):
    if "nc" not in _CACHE:
        _CACHE["nc"] = _build_nc()
    nc = _CACHE["nc"]

    in_maps = _shard_inputs(q, k, v, idx_q, idx_k)
    trace = bool(int(os.environ.get("BSA_TRACE", "0")))
    kwargs = {}
    if trace:
        tmpdir = os.environ.get("BSA_TRACE_DIR") or None
        kwargs = {"trace": True, "tmpdir": tmpdir}
    res = run_bass_kernel_spmd(nc, in_maps, core_ids=list(range(NCORES)),
                               **kwargs)
    if trace:
        print(f"HW exec time: {res.exec_time_ns} ns")
        _CACHE["exec_time_ns"] = res.exec_time_ns

    out = np.zeros((B, H, S, D), np.float32)
    for c in range(NCORES):
        o = np.asarray(res.results[c]["out"], np.float32)   # [3, QH, 129]
        o = o[:, :, :BLOCK] / o[:, :, BLOCK:BLOCK + 1]
        for j, uu in enumerate(range(3 * c, 3 * c + 3)):
            h, hf = uu // 2, uu % 2
            out[0, h, hf * QH:(hf + 1) * QH, :] = o[j]
    return out


# revision 67
# speedup vs baseline: 1.1837x; 1.1837x over previous
"""Adaptive block-sparse attention (train fwd) on 8 Trainium2 NeuronCores.

Reference semantics (B=1, H=12, S=4096, D=128, BLOCK=128, NUM_KEEP=32):
  1. per (b,h): sample 32 tokens/block from q and k (shared intra-block offsets)
  2. pooled attention estimate -> block scores pool[qb, kb]  (32x32 per head)
  3. energy threshold (0.95) on descending-sorted block scores -> keep top-n
     blocks per q-block row, n clipped to [1, 5]
  4. block-sparse attention with that mask.

Sharding: 24 units = (head, half of 2048 q rows); core c owns units 3c..3c+2,
which span exactly heads {floor(3c/2), floor(3c/2)+1}. Each core receives the
2 heads' combined K/V block tensor, per-unit Q^T, and f32 pre-sampled
sq^T/sk^T for the pooling estimate (pooling must stay f32: the rank-5/6 pool
gaps go down to ~1e-5 relative). All pooling, mask generation, top-k
selection and the block-sparse attention run on-device; the host lays out
shards and divides by the returned softmax-denominator column on unshard.

Combined K/V layout per (head-slot j, block b), stride BSC=264 bf16 cols:
  cols 0..127   K^T block   kv[p=d, c]     = k[h, b*128+c, p]
  cols 128..255 V block     kv[p=k, 128+c] = v[h, b*128+p, c]
  col  256      ones        (denominator accumulated by the PV matmul)
  block 32 is all zero: masked slots aim their V offset at it, so exp of the
  (real) scores times zero V and zero ones-col adds nothing to numerator or
  denominator.

Engine plan (per q-block: 5 combined-block gathers + 5 S^T matmuls + one
[128, 640] exp + 5 accumulating PV matmuls + PSUM evacuation):
  SP     gather slot 0 (HWDGE DMA), batched output DMAs, input DMAs
  GpSimd gather slot 1 (SWDGE DMA - data moves on the SDMA engines, so no
         SBUF-port contention with DVE)
  DVE    gather slots 2-4 (dynamic-offset tensor_copy), half the PSUM
         evacuations, pooling reductions + mask ops, the offset transpose
  ACT    exp, plus half the PSUM evacuations
  PE     all matmuls (both attention operands are static slices of the
         gathered combined block, so PE needs no registers at all);
         per-unit offsets reach SP/GpSimd/DVE via one bulk 16-value
         register load per gather engine row

The pooling/mask chain for unit u+1 is emitted interleaved between unit u's
attention q-blocks so its long serial latency hides under attention work.
"""
import os
import sys

sys.path.insert(0, "/opt/trn_rl_repo")

import numpy as np
import ml_dtypes

import concourse.bass as bass
import concourse.bacc as bacc
import concourse.mybir as mybir
from concourse import tile
from concourse.bass_utils import run_bass_kernel_spmd

B, H, S, D = 1, 12, 4096, 128
BLOCK = 128
NUM_KEEP = 32
NB = S // BLOCK            # 32 k-blocks per head
E_THR = 0.95
NCORES = 8
UPC = 3                    # units per core
QH = 2048                  # q rows per unit (half head)
NQB = QH // BLOCK          # 16 q-blocks per unit
NS = 5                     # retained-slot count (MAX_RETAIN)
BSC = 264                  # combined block stride: 128 K + 128 V + 1 ones + 7 pad
HW_ = (NB + 1) * BSC       # combined width per head (incl. zero block)
KVW = 2 * HW_              # two heads
SCALE = float(1.0 / np.sqrt(D))

_CACHE = {}


def _build_nc():
    f32 = mybir.dt.float32
    bf16 = mybir.dt.bfloat16
    i32 = mybir.dt.int32
    u32 = mybir.dt.uint32
    EXP = mybir.ActivationFunctionType.Exp
    OP = mybir.AluOpType
    SP = mybir.EngineType.SP
    ACT = mybir.EngineType.Activation
    DVE = mybir.EngineType.DVE
    POOL = mybir.EngineType.Pool
    PE = mybir.EngineType.PE

    nc = bacc.Bacc("TRN2", target_bir_lowering=False, debug=False,
                   num_devices=NCORES)

    qT_d = nc.dram_tensor("qT", [UPC, D, QH], bf16, kind="ExternalInput")
    kv_d = nc.dram_tensor("kv", [D, KVW], bf16, kind="ExternalInput")
    smp_d = nc.dram_tensor("smp", [UPC, D, (NQB + NB) * NUM_KEEP], f32, kind="ExternalInput")
    qbsel_d = nc.dram_tensor("qbsel", [D, 4 * NQB], f32, kind="ExternalInput")
    misc_d = nc.dram_tensor("misc", [NQB, NS + UPC], f32, kind="ExternalInput")
    out_d = nc.dram_tensor("out", [UPC, QH, BLOCK + 1], f32, kind="ExternalOutput")

    with tile.TileContext(nc) as tc:
        with (
            tc.tile_pool(name="const", bufs=1) as cpool,
            tc.tile_pool(name="unit", bufs=3) as upool,
            tc.tile_pool(name="pwork", bufs=3) as pwork,
            tc.tile_pool(name="mask", bufs=2) as mpool,
            tc.tile_pool(name="kg", bufs=10) as kgpool,
            tc.tile_pool(name="pt", bufs=4) as ptpool,
            tc.tile_pool(name="big", bufs=2, space="PSUM") as bigp,
            tc.tile_pool(name="po", bufs=2, space="PSUM") as pop,
            tc.tile_pool(name="pp", bufs=1, space="PSUM") as ppp,
        ):
            qbsel = cpool.tile([D, 4 * NQB], f32)
            nc.sync.dma_start(qbsel, qbsel_d[:, :])
            misc = cpool.tile([NQB, NS + UPC], f32)
            nc.sync.dma_start(misc, misc_d[:, :])
            iota5 = misc[:, 0:NS]
            cbase = misc[:, NS:NS + UPC]
            zero8 = cpool.tile([NQB, 8], f32)
            nc.vector.memset(zero8, 0.0)
            warm = cpool.tile([NQB, 8], f32)
            nc.scalar.activation(warm, zero8, EXP, scale=1.0)
            kv = cpool.tile([D, KVW], bf16)

            unit_state = {}

            def emit_pool_steps(u):
                """Generator of pooling/mask emission steps for unit u."""
                st = {}
                unit_state[u] = st

                def load():
                    st["qT"] = upool.tile([D, QH], bf16, tag="qT", name="qTu")
                    nc.sync.dma_start(st["qT"], qT_d[u, :, :])
                    smp = upool.tile([D, (NQB + NB) * NUM_KEEP], f32, tag="smp", name="smpu")
                    nc.sync.dma_start(smp, smp_d[u, :, :])
                    st["smp"] = smp
                    st["poolps"] = ppp.tile([NQB, NB], f32, tag="pp", name="poolps")
                yield load

                for t in range(4):
                    for half in range(2):
                        def chunk(t=t, half=half):
                            smp = st["smp"]
                            sqT = smp[:, 0:NQB * NUM_KEEP]
                            skT = smp[:, NQB * NUM_KEEP:(NQB + NB) * NUM_KEEP]
                            ps = bigp.tile([D, 512], f32, tag="big", name="psc")
                            nc.tensor.matmul(
                                ps, sqT[:, t * 128:(t + 1) * 128],
                                skT[:, half * 512:(half + 1) * 512],
                                start=True, stop=True)
                            E = pwork.tile([D, 512], f32, tag="E")
                            nc.scalar.activation(E, ps, EXP, scale=SCALE)
                            if half == 0:
                                st["W"] = pwork.tile([D, NB], f32, tag="W", name="Wt")
                            nc.vector.reduce_sum(
                                st["W"][:, half * 16:(half + 1) * 16],
                                E.rearrange("p (a b) -> p a b", a=16),
                                axis=mybir.AxisListType.X)
                        yield chunk

                    def tfin(t=t):
                        W = st["W"]
                        T = pwork.tile([D, 1], f32, tag="T")
                        nc.vector.reduce_sum(T, W, axis=mybir.AxisListType.X)
                        R = pwork.tile([D, 1], f32, tag="R")
                        nc.vector.reciprocal(R, T)
                        Wn = pwork.tile([D, NB], f32, tag="Wn")
                        nc.vector.tensor_scalar_mul(Wn, W, R[:, 0:1])
                        nc.tensor.matmul(
                            st["poolps"], qbsel[:, t * NQB:(t + 1) * NQB], Wn,
                            start=(t == 0), stop=(t == 3))
                    yield tfin

                def mask_fin():
                    poolps = st["poolps"]
                    pool_sb = mpool.tile([NQB, NB], f32, tag="pool_sb")
                    nc.vector.tensor_copy(pool_sb, poolps)
                    T16 = mpool.tile([NQB, 1], f32, tag="T16")
                    nc.vector.reduce_sum(T16, pool_sb, axis=mybir.AxisListType.X)
                    thr = mpool.tile([NQB, 1], f32, tag="thr")
                    nc.vector.tensor_scalar_mul(thr, T16, E_THR)
                    m8 = mpool.tile([NQB, 8], f32, tag="m8")
                    i8 = mpool.tile([NQB, 8], u32, tag="i8")
                    nc.vector.max_with_indices(m8, i8, pool_sb)
                    cum = mpool.tile([NQB, 8], f32, tag="cum")
                    nc.vector.tensor_tensor_scan(cum, m8, zero8, 0.0,
                                                 OP.add, OP.add)
                    lt = mpool.tile([NQB, 8], f32, tag="lt")
                    nc.vector.tensor_scalar(lt, cum, thr[:, 0:1], None,
                                            op0=OP.is_lt)
                    cnt = mpool.tile([NQB, 1], f32, tag="cnt")
                    nc.vector.reduce_sum(cnt, lt, axis=mybir.AxisListType.X)
                    nkeep = mpool.tile([NQB, 1], f32, tag="nkeep")
                    nc.vector.tensor_scalar(nkeep, cnt, 1.0, float(NS),
                                            op0=OP.max, op1=OP.min)
                    slt = mpool.tile([NQB, NS], f32, tag="slt")
                    nc.vector.tensor_scalar(slt, iota5, nkeep[:, 0:1], None,
                                            op0=OP.is_lt)
                    idx5f = mpool.tile([NQB, NS], f32, tag="idx5f")
                    nc.vector.tensor_copy(idx5f, i8[:, 0:NS])
                    # cofs = 264*slt*(idx-32) + (kbase[u] + 32*264): kept slots
                    # hit the real block, masked ones the all-zero block 32
                    a5 = mpool.tile([NQB, NS], f32, tag="a5")
                    nc.vector.tensor_scalar(a5, idx5f, float(NB), None,
                                            op0=OP.subtract)
                    nc.vector.tensor_mul(a5, a5, slt)
                    cofs = mpool.tile([NQB, NS], f32, tag="cofs")
                    nc.vector.tensor_scalar_mul(cofs, a5, float(BSC))
                    nc.vector.tensor_add(
                        cofs, cofs, cbase[:, u:u + 1].to_broadcast([NQB, NS]))
                    ofs = mpool.tile([32, 32], i32, tag="ofs")
                    nc.vector.memset(ofs, 0)
                    nc.vector.tensor_copy(ofs[0:NQB, 0:NS], cofs)
                    ofsT = mpool.tile([32, 32], i32, tag="ofsT")
                    nc.vector.transpose(ofsT, ofs)
                    kvals = []
                    for s, engs in ((0, [SP]), (1, [POOL]), (2, [DVE]),
                                    (3, [DVE]), (4, [DVE])):
                        _, vv = nc.values_load_multi_w_load_instructions(
                            ofsT[s:s + 1, 0:NQB], engines=engs,
                            min_val=0, max_val=KVW - BSC,
                            skip_runtime_bounds_check=True)
                        kvals.append(vv)
                    st["kvals"] = kvals
                yield mask_fin

            def attn_a(u, qb):
                # gathers + S^T matmuls + exp; PV deferred one q-block so
                # PE's in-order stream never waits on exp
                st = unit_state[u]
                kvals = st["kvals"]
                kgt = []
                for s in range(NS):
                    kg = kgpool.tile([D, BSC], bf16, tag=f"kg{s}")
                    kgt.append(kg)
                    src = kv[:, bass.ds(kvals[s][qb], BSC)]
                    if s == 0:
                        nc.sync.dma_start(kg, src)
                    elif s == 1:
                        nc.gpsimd.dma_start(kg, src)
                    else:
                        nc.vector.tensor_copy(kg, src)
                ps2 = bigp.tile([D, NS * BLOCK], f32, tag="big")
                for s in range(NS):
                    nc.tensor.matmul(
                        ps2[:, s * BLOCK:(s + 1) * BLOCK], kgt[s][:, 0:BLOCK],
                        st["qT"][:, qb * BLOCK:(qb + 1) * BLOCK],
                        start=True, stop=True)
                pT = ptpool.tile([D, NS * BLOCK], bf16, tag="pT")
                nc.scalar.activation(pT, ps2, EXP, scale=SCALE)
                return pT, kgt

            def attn_b(u, qb, pT, kgt):
                st = unit_state[u]
                po = pop.tile([D, BLOCK + 1], f32, tag="po")
                for s in range(NS):
                    nc.tensor.matmul(
                        po, pT[:, s * BLOCK:(s + 1) * BLOCK],
                        kgt[s][:, BLOCK:2 * BLOCK + 1],
                        start=(s == 0), stop=(s == NS - 1))
                j = qb % 4
                if j == 0:
                    st["outsb"] = ptpool.tile([D, 4, BLOCK + 1], f32, tag="outsb", name="outsb")
                nc.vector.tensor_copy(st["outsb"][:, j, :], po)
                if j == 3:
                    nc.sync.dma_start(
                        out_d[u, (qb - 3) * BLOCK:(qb + 1) * BLOCK, :]
                        .rearrange("(a p) c -> p a c", p=BLOCK),
                        st["outsb"])

            # unit 0's pooling/mask runs upfront (its small input DMAs are
            # issued before the big kv load so the pooling chain starts
            # immediately); unit u+1's pooling is interleaved into unit u's
            # attention q-blocks.
            for i, step in enumerate(emit_pool_steps(0)):
                step()
                if i == 0:
                    nc.sync.dma_start(kv, kv_d[:, :])
            for u in range(UPC):
                nxt = iter(emit_pool_steps(u + 1)) if u + 1 < UPC else None
                pend = None
                for qb in range(NQB):
                    a = attn_a(u, qb)
                    if pend is not None:
                        attn_b(u, qb - 1, *pend)
                    pend = a
                    if nxt is not None:
                        for _ in range(2):
                            step = next(nxt, None)
                            if step is not None:
                                step()
                attn_b(u, NQB - 1, *pend)

    nc.compile()
    return nc


def _shard_inputs(q, k, v, idx_q, idx_k):
    """Build the 8 per-core input maps."""
    bf16 = ml_dtypes.bfloat16
    q = np.asarray(q, np.float32)[0]          # [H, S, D]
    k = np.asarray(k, np.float32)[0]
    v = np.asarray(v, np.float32)[0]
    idx_q = np.asarray(idx_q)[0]              # [H, NUM_KEEP]
    idx_k = np.asarray(idx_k)[0]

    qbsel = np.zeros((D, 4 * NQB), np.float32)
    for t in range(4):
        for s in range(D):
            qbsel[s, t * NQB + t * 4 + s // NUM_KEEP] = 1.0
    iota5 = np.tile(np.arange(NS, dtype=np.float32), (NQB, 1))

    in_maps = []
    for c in range(NCORES):
        h_lo = (3 * c) // 2
        units = [(uu // 2, uu % 2) for uu in range(3 * c, 3 * c + 3)]

        # combined K/V blocks, stride BSC; block NB left all-zero
        kvc = np.zeros((D, 2, NB + 1, BSC), np.float32)
        for j, h in enumerate((h_lo, h_lo + 1)):
            kvc[:, j, :NB, :BLOCK] = k[h].reshape(NB, BLOCK, D).transpose(2, 0, 1)
            kvc[:, j, :NB, BLOCK:2 * BLOCK] = (
                v[h].reshape(NB, BLOCK, D).transpose(1, 0, 2))
            kvc[:, j, :NB, 2 * BLOCK] = 1.0
        kvc = kvc.reshape(D, KVW).astype(bf16)

        qT = np.stack([q[h, hf * QH:(hf + 1) * QH].T for h, hf in units]
                      ).astype(bf16)                               # [3, D, QH]

        sqT, skT = [], []
        for h, hf in units:
            sq = q[h, hf * QH:(hf + 1) * QH].reshape(NQB, BLOCK, D)[
                :, idx_q[h], :]                                    # [16, 32, D]
            sqT.append(sq.transpose(2, 0, 1).reshape(D, NQB * NUM_KEEP))
            sk = k[h].reshape(NB, BLOCK, D)[:, idx_k[h], :]        # [32, 32, D]
            skT.append(sk.transpose(2, 0, 1).reshape(D, NB * NUM_KEEP))
        smp = np.concatenate(
            [np.stack(sqT), np.stack(skT)], axis=2).astype(np.float32)

        uslot = np.array([h - h_lo for h, _ in units], np.float32)
        cbase = np.tile(uslot * HW_ + NB * BSC, (NQB, 1)).astype(np.float32)
        misc = np.concatenate([iota5, cbase], axis=1).astype(np.float32)

        in_maps.append({
            "qT": qT, "kv": kvc, "smp": smp, "qbsel": qbsel, "misc": misc,
        })
    return in_maps


def kernel(q, k, v, idx_q, idx_k):
    if "nc" not in _CACHE:
        _CACHE["nc"] = _build_nc()
    nc = _CACHE["nc"]

    in_maps = _shard_inputs(q, k, v, idx_q, idx_k)
    trace = bool(int(os.environ.get("BSA_TRACE", "0")))
    kwargs = {}
    if trace:
        tmpdir = os.environ.get("BSA_TRACE_DIR") or None
        kwargs = {"trace": True, "tmpdir": tmpdir}
    res = run_bass_kernel_spmd(nc, in_maps, core_ids=list(range(NCORES)),
                               **kwargs)
    if trace:
        print(f"HW exec time: {res.exec_time_ns} ns")
        _CACHE["exec_time_ns"] = res.exec_time_ns

    out = np.zeros((B, H, S, D), np.float32)
    for c in range(NCORES):
        o = np.asarray(res.results[c]["out"], np.float32)   # [3, QH, 129]
        o = o[:, :, :BLOCK] / o[:, :, BLOCK:BLOCK + 1]
        for j, uu in enumerate(range(3 * c, 3 * c + 3)):
            h, hf = uu // 2, uu % 2
            out[0, h, hf * QH:(hf + 1) * QH, :] = o[j]
    return out
